# revision 58
# baseline (speedup 1.0000x reference)
"""Trainium2 Bass kernel for nn_EqLayerNodeAttr (gnn message passing).

Strategy:
  - Edges sharded across 8 cores by whole destination-node (col) groups, so
    each core owns a disjoint set of output rows -> no output collectives.
  - The node-feature table (272 bf16 per node) and the MLP weights are
    sharded 8 ways on the host; each core ships 1/8 and the full table is
    rebuilt on device with one AllGather over NeuronLink.  This cuts the
    host->device traffic 8x vs. replicating the table.
  - Within a core, edges are packed into tiles of <=512 edges covering <=64
    distinct destination nodes.  Per tile:
      * src node rows gathered with multi-offset indirect DMAs
      * dst node rows: the <=64 distinct rows are gathered once ("window"),
        then expanded per-edge with a one-hot matmul on the PE
      * per-edge 2x2 rotations on DVE using two strided views of a single
        natural-order (k,m,l) rotation table (no duplicated storage)
      * features transposed to [feat, edge] layout via PE transposes
      * 608->256->192 MLP as bf16 matmuls with fp32 PSUM accumulation
      * messages rotated back per edge, segment-summed over the tile's
        <=64 destinations with a one-hot matmul, and written to a compact
        per-tile output block with a plain DMA (no indirect scatter).
"""

import numpy as np
import ml_dtypes

# ---- problem constants (hardcoded per contract) ----
N = 10000
E = 160000
L = 4
NS, NSA = 64, 16
NR, NRA = 16, 8
DIST = 64
HID = 256
SCAL = NS + NSA            # 80
NREP = NR + NRA            # 24
ROTF = NREP * 2 * L        # 192
FEAT = SCAL + ROTF         # 272
DIN = 2 * FEAT + DIST      # 608
DOUT = NS + NR * 2 * L     # 192
MESHN = 2                  # cores per mesh (one SPMD program + AllGather group)
NMESH = 4                  # independent meshes run concurrently; per-execute
                           # dispatch overhead scales with mesh size but
                           # overlaps almost fully across disjoint meshes
NCORES = MESHN * NMESH     # total cores = edge shards

TP = 512                   # edges per tile
SUB = 128                  # edges per sub-tile
NSUBT = TP // SUB          # 4
W = 64                     # max distinct destination nodes per tile
MW = 41                    # packed metadata words per lane

# allgathered payload layout (bf16 elements, flat)
W1OFF = N * FEAT                       # 2_720_000
W2OFF = W1OFF + 6 * 128 * HID          # + 196_608
B1OFF = W2OFF + 2 * 128 * DOUT         # + 49_152
PAYW = B1OFF + 2 * HID                 # + 512 (b1 as f32 pairs)
WROWS = -(-(PAYW - N * FEAT) // FEAT)  # weight rows
TROWS = -(-(N + WROWS) // MESHN) * MESHN
SROWS = TROWS // MESHN

# packed edges tensor (partition-major so each region loads as ONE big DMA):
#  metaAll: [SUB, T*MW] i32, then distAll: [DIST, T*TP] fp8-e4m3
def _edgew(T):
    return SUB * T * MW + DIST * T * TP // 4

BF16 = ml_dtypes.bfloat16
F8E4 = ml_dtypes.float8_e4m3

# K-chunks of the MLP input (W1 rows reordered to match, see _w1_chunks):
#  c0: dst_rot[0:128]            (featT block 0)
#  c1: dst_rot[128:192] p0:64  | src_rot[128:192] p64:128   (featT block 1)
#  c2: src_rot[0:128]            (featT block 2)
#  c3: dst_scal[0:80]            (sdst tile)
#  c4: src_scal[0:80]            (ssrc tile)
#  c5: dist[0:64]                (dist tile)
KC = [128, 128, 128, SCAL, SCAL, DIST]


def _w1_chunks():
    dst_scal = np.arange(0, 80)
    dst_rot = np.arange(80, 272)
    src_scal = np.arange(272, 352)
    src_rot = np.arange(352, 544)
    dist = np.arange(544, 608)
    return [
        dst_rot[0:128],
        np.concatenate([dst_rot[128:192], src_rot[0:64]]),
        src_rot[64:192],
        dst_scal,
        src_scal,
        dist,
    ]


# --------------------------------------------------------------------------
# host-side sharding / tiling
# --------------------------------------------------------------------------

def _shard_and_tile(row, col):
    """Group edges by destination col; split whole cols across 8 cores with
    balanced edge counts; pack each core's cols into (<=TP edges, <=W cols)
    tiles."""
    order = np.argsort(col, kind="stable")
    scol = col[order]
    uniq, starts = np.unique(scol, return_index=True)
    starts = np.append(starts, len(scol))

    per_core_tiles = [[] for _ in range(NCORES)]
    core_cols = [[] for _ in range(NCORES)]
    target = len(scol) / NCORES
    ci = 0
    for ui in range(len(uniq)):
        lo = starts[ui]
        while ci < NCORES - 1 and lo >= (ci + 1) * target:
            ci += 1
        core_cols[ci].append(ui)

    for c in range(NCORES):
        tiles = []
        cur_e, cur_c = [], []
        for ui in core_cols[c]:
            lo, hi = starts[ui], starts[ui + 1]
            deg = hi - lo
            if deg > TP:
                raise ValueError("col degree exceeds tile capacity")
            if cur_e and (len(cur_e) + deg > TP or len(cur_c) + 1 > W):
                tiles.append((np.array(cur_e, np.int64), np.array(cur_c, np.int64)))
                cur_e, cur_c = [], []
            cur_e.extend(order[lo:hi].tolist())
            cur_c.append(int(uniq[ui]))
        if cur_e:
            tiles.append((np.array(cur_e, np.int64), np.array(cur_c, np.int64)))
        per_core_tiles[c] = tiles
    return per_core_tiles


def _host_prep(inputs):
    x_scalar = np.asarray(inputs["x_scalar"], np.float32)
    x_rot = np.asarray(inputs["x_rot"], np.float32)
    na_scalar = np.asarray(inputs["na_scalar"], np.float32)
    na_rot = np.asarray(inputs["na_rot"], np.float32)
    edge_index = np.asarray(inputs["edge_index"])
    dist_emb = np.asarray(inputs["dist_emb"], np.float32)
    rot = np.asarray(inputs["rot"], np.float32)
    W1 = np.asarray(inputs["W1"], np.float32)
    b1 = np.asarray(inputs["b1"], np.float32)
    W2 = np.asarray(inputs["W2"], np.float32)
    b2 = np.asarray(inputs["b2"], np.float32)

    row = edge_index[0].astype(np.int64)
    col = edge_index[1].astype(np.int64)

    # ---- allgathered payload: node table + weights ----
    xs = np.concatenate([x_scalar, na_scalar], axis=1)               # [N, 80]
    xr = np.concatenate([x_rot, na_rot], axis=1).reshape(N, ROTF)    # [N, 192]
    payload = np.zeros(TROWS * FEAT, BF16)
    payload[: N * FEAT] = (
        np.concatenate([xs, xr], axis=1).astype(BF16).reshape(-1)
    )
    W1c = np.zeros((6, 128, HID), np.float32)
    for c, idx in enumerate(_w1_chunks()):
        W1c[c, : len(idx)] = W1[idx]
    payload[W1OFF:W2OFF] = W1c.astype(BF16).reshape(-1)
    payload[W2OFF:B1OFF] = W2.reshape(2, 128, DOUT).astype(BF16).reshape(-1)
    b1c = np.ascontiguousarray(b1.reshape(2, 128).T)                 # [128, 2] f32
    payload[B1OFF:PAYW] = np.frombuffer(b1c.tobytes(), dtype=BF16)
    shards = payload.reshape(MESHN, SROWS, FEAT)

    per_core_tiles = _shard_and_tile(row, col)
    T = max(len(t) for t in per_core_tiles)

    # rote: natural (k, m, l) order, 16 bf16 per edge
    rote = rot.reshape(-1, 16)

    per_core_inputs = []
    for c in range(NCORES):
        tiles = per_core_tiles[c]
        # packed per-lane metadata words:
        #  0:4 ridx | 4:8 crel | 8:40 rote (4 subs x 16 bf16) | 40 winrows
        meta = np.zeros((T, SUB, MW), np.int32)
        meta[:, :, 4:8] = 127          # crel padding -> no onehot match
        dist = np.zeros((T, DIST, TP), F8E4)
        r_bf = np.zeros((T, SUB, NSUBT * 16), BF16)
        for t in range(T):
            if t >= len(tiles):
                continue
            eids, cols = tiles[t]
            ne = len(eids)
            slot = np.arange(ne)
            lane, s = slot % SUB, slot // SUB
            m = meta[t]
            m[lane, s] = row[eids].astype(np.int32)
            m[lane, 4 + s] = np.searchsorted(cols, col[eids]).astype(np.int32)
            m[: len(cols), 40] = cols.astype(np.int32)
            cidx = (s * 16)[:, None] + np.arange(16)
            r_bf[t, lane[:, None], cidx] = rote[eids].astype(BF16)
            dist[t, :, :ne] = dist_emb[eids].T.astype(F8E4)

        u = r_bf.view(np.uint16).reshape(T, SUB, 32, 2).astype(np.uint32)
        meta[:, :, 8:40] = (u[..., 0] | (u[..., 1] << 16)).view(np.int32)

        edges = np.zeros(_edgew(T), np.int32)
        # metaAll partition-major: [SUB, T, MW]
        edges[: SUB * T * MW] = (
            meta.transpose(1, 0, 2).reshape(-1)
        )
        # distAll partition-major: [DIST, T, TP] fp8 packed into i32 words
        dv = (
            dist.transpose(1, 0, 2).reshape(-1)
            .view(np.uint8).reshape(-1, 4).astype(np.uint32)
        )
        edges[SUB * T * MW:] = (
            dv[:, 0] | (dv[:, 1] << 8) | (dv[:, 2] << 16) | (dv[:, 3] << 24)
        ).view(np.int32)

        per_core_inputs.append(dict(shard=shards[c % MESHN], edges=edges))

    meta_info = dict(per_core_tiles=per_core_tiles, row=row, col=col,
                     rot=rot, b2=b2)
    return per_core_inputs, T, meta_info


def _assemble(results, meta):
    col = meta["col"]
    deg = np.bincount(col, minlength=N)
    out = np.zeros((N, DOUT), np.float32)
    for c, tiles in enumerate(meta["per_core_tiles"]):
        acc = np.asarray(results[c]["acc"], np.float32)   # [W, T*DOUT]
        for t, (eids, cols) in enumerate(tiles):
            out[cols] = acc[: len(cols), t * DOUT:(t + 1) * DOUT]
    out[deg == 0] = 0.0
    b2 = meta["b2"]
    if np.any(b2):
        out[:, :NS] += np.outer(deg, b2[:NS])
        b2r = b2[NS:].reshape(NR, L, 2)
        rot = meta["rot"]
        corr = np.einsum("jkm,ekml->ejkl", b2r, rot).reshape(E, NR * 2 * L)
        np.add.at(out[:, NS:], col, corr)
    return out


# --------------------------------------------------------------------------
# device program
# --------------------------------------------------------------------------

def _build_program(T, sim_nocc=False, mesh_index=0):
    from concourse import bacc, mybir
    import concourse.tile as tile
    from concourse.bass import IndirectOffsetOnAxis
    from concourse.masks import make_identity

    f32 = mybir.dt.float32
    bf16 = mybir.dt.bfloat16
    f8e4 = mybir.dt.float8e4
    i32 = mybir.dt.int32
    AL = mybir.AluOpType
    ACTF = mybir.ActivationFunctionType

    nc = bacc.Bacc("TRN2", target_bir_lowering=False, debug=False)

    shard_rows = TROWS if sim_nocc else SROWS
    d_shard = nc.dram_tensor(
        "shard", [shard_rows, FEAT], bf16, kind="ExternalInput"
    ).ap()
    d_edges = nc.dram_tensor("edges", [_edgew(T)], i32, kind="ExternalInput").ap()
    d_acc = nc.dram_tensor("acc", [W, T * DOUT], bf16, kind="ExternalOutput").ap()

    with tile.TileContext(nc) as tc:
        with (
            tc.tile_pool(name="dram", bufs=1, space="DRAM") as dpool,
            tc.tile_pool(name="const", bufs=1) as cpool,
            tc.tile_pool(name="sb", bufs=3) as pool,
            tc.tile_pool(name="sb3", bufs=4) as pool3,
            tc.tile_pool(name="ph", bufs=2, space="PSUM") as pph,
            tc.tile_pool(name="po", bufs=1, space="PSUM") as ppo,
            tc.tile_pool(name="ptr", bufs=3, space="PSUM") as ptr,
            tc.tile_pool(name="px", bufs=1, space="PSUM") as px,
            tc.tile_pool(name="psc", bufs=1, space="PSUM") as psc,
        ):
            # ---- allgather the node/weight payload ----
            if sim_nocc:
                ob = dpool.tile([TROWS, FEAT], bf16)
                nc.gpsimd.dma_start(ob[:], d_shard[:])
            else:
                ib = dpool.tile([SROWS, FEAT], bf16)
                ob = dpool.tile([TROWS, FEAT], bf16)
                nc.gpsimd.dma_start(ib[:], d_shard[:])
                base = mesh_index * MESHN
                nc.gpsimd.collective_compute(
                    "AllGather",
                    mybir.AluOpType.bypass,
                    replica_groups=[list(range(base, base + MESHN))],
                    ins=[ib.opt()],
                    outs=[ob.opt()],
                )
            wflat = ob[:].rearrange("r w -> (r w)")

            # ---- constants ----
            ident = cpool.tile([128, 128], bf16)
            make_identity(nc, ident[:])
            iota = cpool.tile([128, W], i32)
            nc.gpsimd.iota(iota[:], pattern=[[1, W]], base=0, channel_multiplier=0)
            w1sb = cpool.tile([128, 6 * HID], bf16)
            nc.sync.dma_start(
                out=w1sb[:].rearrange("p (c w) -> p c w", c=6),
                in_=wflat[W1OFF:W2OFF].rearrange("(c p w) -> p c w", p=128, w=HID),
            )
            w2sb = cpool.tile([128, 2 * DOUT], bf16)
            nc.sync.dma_start(
                out=w2sb[:].rearrange("p (c w) -> p c w", c=2),
                in_=wflat[W2OFF:B1OFF].rearrange("(c p w) -> p c w", p=128, w=DOUT),
            )
            b1sb = cpool.tile([128, 2], f32)
            nc.sync.dma_start(
                out=b1sb[:],
                in_=wflat[B1OFF:PAYW].rearrange("(p w) -> p w", p=128).bitcast(f32),
            )

            # ---- whole-run resident loads: meta, dist; resident output ----
            # chunked across queues so tile 0 starts early and transfers
            # overlap the pipeline
            meta_all = cpool.tile([SUB, T * MW], i32)
            meta_src = d_edges[0:SUB * T * MW].rearrange(
                "(p t w) -> p t w", p=SUB, w=MW
            )
            mchunk = -(-T // 4)
            for k in range(0, T, mchunk):
                ke = min(k + mchunk, T)
                nc.sync.dma_start(
                    out=meta_all[:].rearrange("p (t w) -> p t w", w=MW)[:, k:ke],
                    in_=meta_src[:, k:ke],
                )
            dist_all = cpool.tile([DIST, T * TP], f8e4)
            dist_src = (
                d_edges[SUB * T * MW:]
                .rearrange("(p w) -> p w", p=DIST)
                .bitcast(f8e4)
                .rearrange("p (t e) -> p t e", e=TP)
            )
            dchunk = -(-T // 8)
            for k in range(0, T, dchunk):
                ke = min(k + dchunk, T)
                nc.scalar.dma_start(
                    out=dist_all[:].rearrange("p (t e) -> p t e", e=TP)[:, k:ke],
                    in_=dist_src[:, k:ke],
                )
            out_all = cpool.tile([W, T * DOUT], bf16)
            OCH = 8  # output store chunk (tiles)

            def rot3(x_jkm, rote16, out_jkl, nj, eng):
                """out[j,k,l] = sum_m x[j,k,m] * rote[k,?,?] with the rote
                view supplied per direction.  x_jkm: AP [SUB, nj*L*2]
                (j,k,m); rote16: AP [SUB, 16] natural (k,m,l);
                out_jkl: AP [SUB, nj*L*2]."""
                t0 = pool3.tile([SUB, NREP * L * 2], bf16, tag="rt0")
                t1 = pool3.tile([SUB, NREP * L * 2], bf16, tag="rt1")
                xv = x_jkm.rearrange("p (j k m) -> p j k m", j=nj, k=L, m=2)
                # fwd: in1_m[p,k,l] = rote[k*4 + l*2 + m]  (strides k:4, l:2)
                rv = rote16.rearrange("p (k l m) -> p k l m", k=L, l=2, m=2)
                for m in range(2):
                    eng.tensor_tensor(
                        out=(t0 if m == 0 else t1)[:, : nj * L * 2].rearrange(
                            "p (j k l) -> p j k l", j=nj, k=L, l=2
                        ),
                        in0=xv[:, :, :, m : m + 1].broadcast_to([SUB, nj, L, 2]),
                        in1=rv[:, :, :, m].unsqueeze(1).broadcast_to([SUB, nj, L, 2]),
                        op=AL.mult,
                    )
                eng.tensor_tensor(
                    out=out_jkl,
                    in0=t0[:, : nj * L * 2],
                    in1=t1[:, : nj * L * 2],
                    op=AL.add,
                )

            def rot3_back(m_jkm, rote16, out_jkl, eng):
                """out[j,k,l] = sum_m msg[j,k,m] * rote[k,m,l] (natural)."""
                nj = NR
                t0 = pool3.tile([SUB, NR * L * 2], bf16, tag="bt0")
                t1 = pool3.tile([SUB, NR * L * 2], bf16, tag="bt1")
                xv = m_jkm.rearrange("p (j k m) -> p j k m", j=nj, k=L, m=2)
                rv = rote16.rearrange("p (k m l) -> p k m l", k=L, m=2, l=2)
                for m in range(2):
                    eng.tensor_tensor(
                        out=(t0 if m == 0 else t1)[:].rearrange(
                            "p (j k l) -> p j k l", j=nj, k=L, l=2
                        ),
                        in0=xv[:, :, :, m : m + 1].broadcast_to([SUB, nj, L, 2]),
                        in1=rv[:, :, m, :].unsqueeze(1).broadcast_to([SUB, nj, L, 2]),
                        op=AL.mult,
                    )
                eng.tensor_tensor(out=out_jkl, in0=t0[:], in1=t1[:], op=AL.add)

            def emit_front(t):
                # ---- per-tile views of the resident tiles ----
                meta = meta_all[:, t * MW:(t + 1) * MW]
                dist_sb = pool.tile([DIST, TP], bf16, tag="distbf")
                nc.scalar.activation(
                    out=dist_sb[:], in_=dist_all[:, t * TP:(t + 1) * TP],
                    func=ACTF.Copy,
                )
                ridx = meta[:, 0:4]
                crel = meta[:, 4:8]
                rote = meta[:, 8:40].bitcast(bf16)       # [128, 64]
                winr = meta[0:W, 40:41]

                # ---- one-hots (independent of gathers) ----
                onehot_e = pool.tile([SUB, NSUBT * W], bf16)
                onehot_w = pool.tile([W, TP], bf16)
                p_oh = ptr.tile([W, TP], bf16, tag="ptrans")
                nc.vector.tensor_tensor(
                    out=onehot_e[:].rearrange("p (s w) -> p s w", s=NSUBT),
                    in0=crel[:, 0:NSUBT].unsqueeze(-1).broadcast_to([SUB, NSUBT, W]),
                    in1=iota[:, :].unsqueeze(1).broadcast_to([SUB, NSUBT, W]),
                    op=AL.is_equal,
                )
                for s in range(NSUBT):
                    nc.tensor.transpose(
                        out=p_oh[:, s * SUB:(s + 1) * SUB],
                        in_=onehot_e[:, s * W:(s + 1) * W],
                        identity=ident[:],
                    )
                nc.scalar.activation(out=onehot_w[:], in_=p_oh[:], func=ACTF.Copy)

                # ---- gathers (from the allgathered table) ----
                win = pool.tile([W, FEAT], bf16)
                nc.gpsimd.indirect_dma_start(
                    out=win[:],
                    out_offset=None,
                    in_=ob[:],
                    in_offset=IndirectOffsetOnAxis(ap=winr, axis=0),
                )
                src_g = pool.tile([SUB, NSUBT * FEAT], bf16)
                for s in range(NSUBT):
                    nc.gpsimd.indirect_dma_start(
                        out=src_g[:, s * FEAT:(s + 1) * FEAT],
                        out_offset=None,
                        in_=ob[:],
                        in_offset=IndirectOffsetOnAxis(ap=ridx[:, s:s + 1], axis=0),
                    )

                featT = pool.tile([128, 3 * TP], bf16)
                sdst = pool.tile([SCAL, TP], bf16)
                ssrc = pool.tile([SCAL, TP], bf16)

                for s in range(NSUBT):
                    cL = s * SUB
                    rfs = rote[:, s * 16:(s + 1) * 16]

                    # ---- dst rot features: expand + rotate ----
                    p_x1 = px.tile([SUB, ROTF], f32, tag="px")
                    nc.tensor.matmul(
                        out=p_x1[:],
                        lhsT=onehot_w[:, cL:cL + SUB],
                        rhs=win[:, SCAL:FEAT],
                        start=True,
                        stop=True,
                    )
                    # xrot = [dst_rot 192 | src_rot 192] in one tile so the
                    # three 128-col transposes cover both
                    sg = src_g[:, s * FEAT:(s + 1) * FEAT]
                    xrot = pool3.tile([SUB, 2 * ROTF], bf16, tag="xrot")
                    rot3(p_x1[:], rfs, xrot[:, 0:ROTF], NREP, nc.vector)
                    rot3(sg[:, SCAL:FEAT], rfs, xrot[:, ROTF:2 * ROTF], NREP,
                         nc.vector)

                    # ---- transposes into chunk tiles ----
                    ptn = ptr.tile([128, 512], bf16, tag="ptrans")
                    for bb in range(3):
                        nc.tensor.transpose(
                            out=ptn[:, bb * 128:(bb + 1) * 128],
                            in_=xrot[:, bb * 128:(bb + 1) * 128],
                            identity=ident[:],
                        )
                    nc.tensor.transpose(
                        out=ptn[0:SCAL, 384:512], in_=sg[:, 0:SCAL], identity=ident[:]
                    )
                    # merged copy of the three 128-part sections -> featT blocks
                    nc.scalar.activation(
                        out=featT[:].rearrange("p (c e) -> p c e", c=3, e=TP)[
                            :, :, cL:cL + SUB
                        ],
                        in_=ptn[:, 0:384].rearrange("p (c e) -> p c e", c=3, e=SUB),
                        func=ACTF.Copy,
                    )
                    nc.scalar.activation(
                        out=ssrc[:, cL:cL + SUB], in_=ptn[0:SCAL, 384:512],
                        func=ACTF.Copy,
                    )

                # ---- dst scalar expand (once per tile) ----
                p_x2 = pph.tile([SCAL, TP], f32, tag="ph")
                nc.tensor.matmul(
                    out=p_x2[:],
                    lhsT=win[:, 0:SCAL],
                    rhs=onehot_w[:],
                    start=True,
                    stop=True,
                )
                nc.scalar.activation(out=sdst[:], in_=p_x2[:], func=ACTF.Copy)

                # ---- MLP layer 1 + relu ----
                rhs_chunks = [
                    featT[:, 0:TP], featT[:, TP:2 * TP], featT[:, 2 * TP:3 * TP],
                    sdst[:], ssrc[:], dist_sb[:],
                ]
                hT = pool.tile([128, 2 * TP], bf16)
                for hh in range(2):
                    p_h = pph.tile([128, TP], f32, tag="ph")
                    for c in range(6):
                        nc.tensor.matmul(
                            out=p_h[:],
                            lhsT=w1sb[0:KC[c], c * HID + hh * 128:c * HID + (hh + 1) * 128],
                            rhs=rhs_chunks[c][0:KC[c], :],
                            start=(c == 0),
                            stop=(c == 5),
                        )
                    nc.scalar.activation(
                        out=hT[:, hh * TP:(hh + 1) * TP],
                        in_=p_h[:],
                        func=ACTF.Relu,
                        bias=b1sb[:, hh:hh + 1],
                    )

                # ---- MLP layer 2 (2 partition chunks of 128/64) ----
                msgT = []
                for dd, (d0, dw) in enumerate([(0, 128), (128, 64)]):
                    p_o = ppo.tile([dw, TP], f32, tag="po")
                    for hh in range(2):
                        nc.tensor.matmul(
                            out=p_o[:],
                            lhsT=w2sb[:, hh * DOUT + d0:hh * DOUT + d0 + dw],
                            rhs=hT[:, hh * TP:(hh + 1) * TP],
                            start=(hh == 0),
                            stop=(hh == 1),
                        )
                    mt = pool.tile([dw, TP], bf16, tag=f"msgT{dd}")
                    if dd == 0:
                        nc.vector.tensor_copy(out=mt[:], in_=p_o[:])
                    else:
                        nc.scalar.activation(out=mt[:], in_=p_o[:], func=ACTF.Copy)
                    msgT.append(mt)

                return dict(rote=rote, onehot_e=onehot_e, msgT=msgT, t=t)

            def emit_back(st):
                rote = st["rote"]
                onehot_e = st["onehot_e"]
                msgT = st["msgT"]
                t = st["t"]
                # ---- back-rotation + segment sum ----
                p_sc = psc.tile([W, DOUT], f32, tag="psc")
                for s in range(NSUBT):
                    cL = s * SUB
                    rbs = rote[:, s * 16:(s + 1) * 16]
                    p_m = ptr.tile([128, DOUT], bf16, tag="ptrans")
                    nc.tensor.transpose(
                        out=p_m[:, 0:128], in_=msgT[0][:, cL:cL + SUB],
                        identity=ident[:],
                    )
                    nc.tensor.transpose(
                        out=p_m[:, 128:192], in_=msgT[1][:, cL:cL + SUB],
                        identity=ident[0:64, 0:64],
                    )
                    out_sb = pool3.tile([SUB, DOUT], bf16, tag="outsb")
                    nc.scalar.activation(
                        out=out_sb[:, 0:NS], in_=p_m[:, 0:NS], func=ACTF.Copy
                    )
                    rot3_back(p_m[:, NS:DOUT], rbs, out_sb[:, NS:DOUT], nc.vector)
                    nc.tensor.matmul(
                        out=p_sc[:],
                        lhsT=onehot_e[:, s * W:(s + 1) * W],
                        rhs=out_sb[:],
                        start=(s == 0),
                        stop=(s == NSUBT - 1),
                    )
                nc.scalar.activation(
                    out=out_all[:, t * DOUT:(t + 1) * DOUT], in_=p_sc[:],
                    func=ACTF.Copy,
                )
                # flush finished output chunk while the pipeline continues
                if (t + 1) % OCH == 0 or t == T - 1:
                    k = (t // OCH) * OCH
                    nc.sync.dma_start(
                        out=d_acc[:, k * DOUT:(t + 1) * DOUT],
                        in_=out_all[:, k * DOUT:(t + 1) * DOUT],
                    )

            # software pipeline: emit front(t+1) before back(t) so the
            # scheduler interleaves t+1's gathers/rotations with t's MLP
            st = emit_front(0)
            for t in range(1, T):
                st_next = emit_front(t)
                emit_back(st)
                st = st_next
            emit_back(st)

    nc.compile()
    return nc


_PROGRAM_CACHE = {}


def _get_program(T, mesh_index=0):
    key = (T, mesh_index)
    if key not in _PROGRAM_CACHE:
        _PROGRAM_CACHE[key] = _build_program(T, mesh_index=mesh_index)
    return _PROGRAM_CACHE[key]


class _PjrtExec:
    """Persistent jitted SPMD executables for one Bass program, run as NMESH
    concurrent disjoint meshes of MESHN cores each (axon/PJRT).  Per-execute
    dispatch overhead overlaps almost fully across the meshes."""

    def __init__(self, ncs):
        import jax
        from jax.sharding import Mesh, PartitionSpec, NamedSharding
        from jax.experimental.shard_map import shard_map
        import concourse.mybir as mybir
        from concourse.bass2jax import (
            _bass_exec_p,
            install_neuronx_cc_hook,
            partition_id_tensor,
        )

        install_neuronx_cc_hook()
        self.ncs = ncs
        nc0 = ncs[0]
        partition_name = (
            nc0.partition_id_tensor.name if nc0.partition_id_tensor else None
        )
        in_names, out_names, out_avals, zero_shapes = [], [], [], []
        for alloc in nc0.m.functions[0].allocations:
            if not isinstance(alloc, mybir.MemoryLocationSet):
                continue
            name = alloc.memorylocations[0].name
            if alloc.kind == "ExternalInput":
                if name != partition_name:
                    in_names.append(name)
            elif alloc.kind == "ExternalOutput":
                shape = tuple(alloc.tensor_shape)
                dtype = mybir.dt.np(alloc.dtype)
                out_names.append(name)
                out_avals.append(jax.core.ShapedArray(shape, dtype))
                zero_shapes.append((shape, dtype))
        self.in_names = in_names
        self.out_names = out_names
        self.out_avals = out_avals
        self.zero_shapes = zero_shapes
        n_params, n_outs = len(in_names), len(out_names)
        all_names = in_names + out_names
        if partition_name is not None:
            all_names.append(partition_name)
        donate = tuple(range(n_params, n_params + n_outs))

        def make_body(nc):
            def _body(*args):
                operands = list(args)
                if partition_name is not None:
                    operands.append(partition_id_tensor())
                outs = _bass_exec_p.bind(
                    *operands,
                    out_avals=tuple(out_avals),
                    in_names=tuple(all_names),
                    out_names=tuple(out_names),
                    lowering_input_output_aliases=(),
                    sim_require_finite=True,
                    sim_require_nnan=True,
                    nc=nc,
                )
                return tuple(outs)

            return _body

        devices = jax.devices()
        self.fns, self.shardings = [], []
        for mi in range(NMESH):
            mesh = Mesh(
                np.asarray(devices[mi * MESHN:(mi + 1) * MESHN]), ("core",)
            )
            self.fns.append(
                jax.jit(
                    shard_map(
                        make_body(ncs[mi]),
                        mesh=mesh,
                        in_specs=(PartitionSpec("core",),) * (n_params + n_outs),
                        out_specs=(PartitionSpec("core",),) * n_outs,
                        check_rep=False,
                    ),
                    donate_argnums=donate,
                    keep_unused=True,
                )
            )
            self.shardings.append(NamedSharding(mesh, PartitionSpec("core")))

    def stage_inputs(self, per_core_inputs):
        import jax

        staged = []
        for mi in range(NMESH):
            cores = range(mi * MESHN, (mi + 1) * MESHN)
            concat_in = [
                np.concatenate(
                    [np.asarray(per_core_inputs[c][n]) for c in cores], axis=0
                )
                for n in self.in_names
            ]
            staged.append(
                [jax.device_put(a, self.shardings[mi]) for a in concat_in]
            )
        return staged

    def fresh_zeros(self):
        return [
            [
                np.zeros((MESHN * s[0], *s[1:]), d)
                for (s, d) in self.zero_shapes
            ]
            for _ in range(NMESH)
        ]

    def put_zeros(self, zeros):
        import jax

        return [
            [jax.device_put(z, self.shardings[mi]) for z in zeros[mi]]
            for mi in range(NMESH)
        ]

    def launch(self, staged, zeros_dev):
        return [
            self.fns[mi](*staged[mi], *zeros_dev[mi]) for mi in range(NMESH)
        ]

    def run(self, staged, zeros):
        import jax

        outs = self.launch(staged, self.put_zeros(zeros))
        jax.block_until_ready(outs)
        return outs

    def results(self, outs):
        res = []
        for c in range(NCORES):
            mi, j = divmod(c, MESHN)
            res.append(
                {
                    n: np.asarray(outs[mi][i]).reshape(
                        MESHN, *self.out_avals[i].shape
                    )[j]
                    for i, n in enumerate(self.out_names)
                }
            )
        return res


_EXEC_CACHE = {}


def _get_exec(T):
    if T not in _EXEC_CACHE:
        _EXEC_CACHE[T] = _PjrtExec(
            [_get_program(T, mesh_index=mi) for mi in range(NMESH)]
        )
    return _EXEC_CACHE[T]


def kernel(**inputs):
    per_core_inputs, T, meta = _host_prep(inputs)
    ex = _get_exec(T)
    staged = ex.stage_inputs(per_core_inputs)
    outs = ex.run(staged, ex.fresh_zeros())
    return _assemble(ex.results(outs), meta)


# revision 59
# speedup vs baseline: 1.6853x; 1.6853x over previous
"""Trainium2 Bass kernel for nn_EqLayerNodeAttr (gnn message passing).

Strategy:
  - Edges sharded across 8 cores by whole destination-node (col) groups, so
    each core owns a disjoint set of output rows -> no output collectives.
  - The node-feature table (272 bf16 per node) and the MLP weights are
    sharded 8 ways on the host; each core ships 1/8 and the full table is
    rebuilt on device with one AllGather over NeuronLink.  This cuts the
    host->device traffic 8x vs. replicating the table.
  - Within a core, edges are packed into tiles of <=512 edges covering <=64
    distinct destination nodes.  Per tile:
      * src node rows gathered with multi-offset indirect DMAs
      * dst node rows: the <=64 distinct rows are gathered once ("window"),
        then expanded per-edge with a one-hot matmul on the PE
      * per-edge 2x2 rotations on DVE using two strided views of a single
        natural-order (k,m,l) rotation table (no duplicated storage)
      * features transposed to [feat, edge] layout via PE transposes
      * 608->256->192 MLP as bf16 matmuls with fp32 PSUM accumulation
      * messages rotated back per edge, segment-summed over the tile's
        <=64 destinations with a one-hot matmul, and written to a compact
        per-tile output block with a plain DMA (no indirect scatter).
"""

import numpy as np
import ml_dtypes

# ---- problem constants (hardcoded per contract) ----
N = 10000
E = 160000
L = 4
NS, NSA = 64, 16
NR, NRA = 16, 8
DIST = 64
HID = 256
SCAL = NS + NSA            # 80
NREP = NR + NRA            # 24
ROTF = NREP * 2 * L        # 192
FEAT = SCAL + ROTF         # 272
DIN = 2 * FEAT + DIST      # 608
DOUT = NS + NR * 2 * L     # 192
MESHN = 4                  # cores per mesh (one SPMD program + AllGather group)
NMESH = 2                  # independent meshes run concurrently; per-execute
                           # dispatch overhead scales with mesh size but
                           # overlaps almost fully across disjoint meshes
NCORES = MESHN * NMESH     # total cores = edge shards

TP = 512                   # edges per tile
SUB = 128                  # edges per sub-tile
NSUBT = TP // SUB          # 4
W = 64                     # max distinct destination nodes per tile
MW = 41                    # packed metadata words per lane

# allgathered payload layout (bf16 elements, flat)
W1OFF = N * FEAT                       # 2_720_000
W2OFF = W1OFF + 6 * 128 * HID          # + 196_608
B1OFF = W2OFF + 2 * 128 * DOUT         # + 49_152
PAYW = B1OFF + 2 * HID                 # + 512 (b1 as f32 pairs)
WROWS = -(-(PAYW - N * FEAT) // FEAT)  # weight rows
TROWS = -(-(N + WROWS) // MESHN) * MESHN
SROWS = TROWS // MESHN

# packed edges tensor (partition-major so each region loads as ONE big DMA):
#  metaAll: [SUB, T*MW] i32, then distAll: [DIST, T*TP] fp8-e4m3
def _edgew(T):
    return SUB * T * MW + DIST * T * TP // 4

BF16 = ml_dtypes.bfloat16
F8E4 = ml_dtypes.float8_e4m3

# K-chunks of the MLP input (W1 rows reordered to match, see _w1_chunks):
#  c0: dst_rot[0:128]            (featT block 0)
#  c1: dst_rot[128:192] p0:64  | src_rot[128:192] p64:128   (featT block 1)
#  c2: src_rot[0:128]            (featT block 2)
#  c3: dst_scal[0:80]            (sdst tile)
#  c4: src_scal[0:80]            (ssrc tile)
#  c5: dist[0:64]                (dist tile)
KC = [128, 128, 128, SCAL, SCAL, DIST]


def _w1_chunks():
    dst_scal = np.arange(0, 80)
    dst_rot = np.arange(80, 272)
    src_scal = np.arange(272, 352)
    src_rot = np.arange(352, 544)
    dist = np.arange(544, 608)
    return [
        dst_rot[0:128],
        np.concatenate([dst_rot[128:192], src_rot[0:64]]),
        src_rot[64:192],
        dst_scal,
        src_scal,
        dist,
    ]


# --------------------------------------------------------------------------
# host-side sharding / tiling
# --------------------------------------------------------------------------

def _shard_and_tile(row, col):
    """Group edges by destination col; split whole cols across 8 cores with
    balanced edge counts; pack each core's cols into (<=TP edges, <=W cols)
    tiles."""
    order = np.argsort(col, kind="stable")
    scol = col[order]
    uniq, starts = np.unique(scol, return_index=True)
    starts = np.append(starts, len(scol))

    per_core_tiles = [[] for _ in range(NCORES)]
    core_cols = [[] for _ in range(NCORES)]
    target = len(scol) / NCORES
    ci = 0
    for ui in range(len(uniq)):
        lo = starts[ui]
        while ci < NCORES - 1 and lo >= (ci + 1) * target:
            ci += 1
        core_cols[ci].append(ui)

    for c in range(NCORES):
        tiles = []
        cur_e, cur_c = [], []
        for ui in core_cols[c]:
            lo, hi = starts[ui], starts[ui + 1]
            deg = hi - lo
            if deg > TP:
                raise ValueError("col degree exceeds tile capacity")
            if cur_e and (len(cur_e) + deg > TP or len(cur_c) + 1 > W):
                tiles.append((np.array(cur_e, np.int64), np.array(cur_c, np.int64)))
                cur_e, cur_c = [], []
            cur_e.extend(order[lo:hi].tolist())
            cur_c.append(int(uniq[ui]))
        if cur_e:
            tiles.append((np.array(cur_e, np.int64), np.array(cur_c, np.int64)))
        per_core_tiles[c] = tiles
    return per_core_tiles


def _host_prep(inputs):
    x_scalar = np.asarray(inputs["x_scalar"], np.float32)
    x_rot = np.asarray(inputs["x_rot"], np.float32)
    na_scalar = np.asarray(inputs["na_scalar"], np.float32)
    na_rot = np.asarray(inputs["na_rot"], np.float32)
    edge_index = np.asarray(inputs["edge_index"])
    dist_emb = np.asarray(inputs["dist_emb"], np.float32)
    rot = np.asarray(inputs["rot"], np.float32)
    W1 = np.asarray(inputs["W1"], np.float32)
    b1 = np.asarray(inputs["b1"], np.float32)
    W2 = np.asarray(inputs["W2"], np.float32)
    b2 = np.asarray(inputs["b2"], np.float32)

    row = edge_index[0].astype(np.int64)
    col = edge_index[1].astype(np.int64)

    # ---- allgathered payload: node table + weights ----
    xs = np.concatenate([x_scalar, na_scalar], axis=1)               # [N, 80]
    xr = np.concatenate([x_rot, na_rot], axis=1).reshape(N, ROTF)    # [N, 192]
    payload = np.zeros(TROWS * FEAT, BF16)
    payload[: N * FEAT] = (
        np.concatenate([xs, xr], axis=1).astype(BF16).reshape(-1)
    )
    W1c = np.zeros((6, 128, HID), np.float32)
    for c, idx in enumerate(_w1_chunks()):
        W1c[c, : len(idx)] = W1[idx]
    payload[W1OFF:W2OFF] = W1c.astype(BF16).reshape(-1)
    payload[W2OFF:B1OFF] = W2.reshape(2, 128, DOUT).astype(BF16).reshape(-1)
    b1c = np.ascontiguousarray(b1.reshape(2, 128).T)                 # [128, 2] f32
    payload[B1OFF:PAYW] = np.frombuffer(b1c.tobytes(), dtype=BF16)
    shards = payload.reshape(MESHN, SROWS, FEAT)

    per_core_tiles = _shard_and_tile(row, col)
    T = max(len(t) for t in per_core_tiles)

    # rote: natural (k, m, l) order, 16 bf16 per edge
    rote = rot.reshape(-1, 16)

    per_core_inputs = []
    for c in range(NCORES):
        tiles = per_core_tiles[c]
        # packed per-lane metadata words:
        #  0:4 ridx | 4:8 crel | 8:40 rote (4 subs x 16 bf16) | 40 winrows
        meta = np.zeros((T, SUB, MW), np.int32)
        meta[:, :, 4:8] = 127          # crel padding -> no onehot match
        dist = np.zeros((T, DIST, TP), F8E4)
        r_bf = np.zeros((T, SUB, NSUBT * 16), BF16)
        for t in range(T):
            if t >= len(tiles):
                continue
            eids, cols = tiles[t]
            ne = len(eids)
            slot = np.arange(ne)
            lane, s = slot % SUB, slot // SUB
            m = meta[t]
            m[lane, s] = row[eids].astype(np.int32)
            m[lane, 4 + s] = np.searchsorted(cols, col[eids]).astype(np.int32)
            m[: len(cols), 40] = cols.astype(np.int32)
            cidx = (s * 16)[:, None] + np.arange(16)
            r_bf[t, lane[:, None], cidx] = rote[eids].astype(BF16)
            dist[t, :, :ne] = dist_emb[eids].T.astype(F8E4)

        u = r_bf.view(np.uint16).reshape(T, SUB, 32, 2).astype(np.uint32)
        meta[:, :, 8:40] = (u[..., 0] | (u[..., 1] << 16)).view(np.int32)

        edges = np.zeros(_edgew(T), np.int32)
        # metaAll partition-major: [SUB, T, MW]
        edges[: SUB * T * MW] = (
            meta.transpose(1, 0, 2).reshape(-1)
        )
        # distAll partition-major: [DIST, T, TP] fp8 packed into i32 words
        dv = (
            dist.transpose(1, 0, 2).reshape(-1)
            .view(np.uint8).reshape(-1, 4).astype(np.uint32)
        )
        edges[SUB * T * MW:] = (
            dv[:, 0] | (dv[:, 1] << 8) | (dv[:, 2] << 16) | (dv[:, 3] << 24)
        ).view(np.int32)

        per_core_inputs.append(dict(shard=shards[c % MESHN], edges=edges))

    meta_info = dict(per_core_tiles=per_core_tiles, row=row, col=col,
                     rot=rot, b2=b2)
    return per_core_inputs, T, meta_info


def _assemble(results, meta):
    col = meta["col"]
    deg = np.bincount(col, minlength=N)
    out = np.zeros((N, DOUT), np.float32)
    for c, tiles in enumerate(meta["per_core_tiles"]):
        acc = np.asarray(results[c]["acc"], np.float32)   # [W, T*DOUT]
        for t, (eids, cols) in enumerate(tiles):
            out[cols] = acc[: len(cols), t * DOUT:(t + 1) * DOUT]
    out[deg == 0] = 0.0
    b2 = meta["b2"]
    if np.any(b2):
        out[:, :NS] += np.outer(deg, b2[:NS])
        b2r = b2[NS:].reshape(NR, L, 2)
        rot = meta["rot"]
        corr = np.einsum("jkm,ekml->ejkl", b2r, rot).reshape(E, NR * 2 * L)
        np.add.at(out[:, NS:], col, corr)
    return out


# --------------------------------------------------------------------------
# device program
# --------------------------------------------------------------------------

def _build_program(T, sim_nocc=False, mesh_index=0):
    from concourse import bacc, mybir
    import concourse.tile as tile
    from concourse.bass import IndirectOffsetOnAxis
    from concourse.masks import make_identity

    f32 = mybir.dt.float32
    bf16 = mybir.dt.bfloat16
    f8e4 = mybir.dt.float8e4
    i32 = mybir.dt.int32
    AL = mybir.AluOpType
    ACTF = mybir.ActivationFunctionType

    nc = bacc.Bacc("TRN2", target_bir_lowering=False, debug=False)

    shard_rows = TROWS if sim_nocc else SROWS
    d_shard = nc.dram_tensor(
        "shard", [shard_rows, FEAT], bf16, kind="ExternalInput"
    ).ap()
    d_edges = nc.dram_tensor("edges", [_edgew(T)], i32, kind="ExternalInput").ap()
    d_acc = nc.dram_tensor("acc", [W, T * DOUT], bf16, kind="ExternalOutput").ap()

    with tile.TileContext(nc) as tc:
        with (
            tc.tile_pool(name="dram", bufs=1, space="DRAM") as dpool,
            tc.tile_pool(name="const", bufs=1) as cpool,
            tc.tile_pool(name="sb", bufs=3) as pool,
            tc.tile_pool(name="sb3", bufs=4) as pool3,
            tc.tile_pool(name="ph", bufs=2, space="PSUM") as pph,
            tc.tile_pool(name="po", bufs=1, space="PSUM") as ppo,
            tc.tile_pool(name="ptr", bufs=3, space="PSUM") as ptr,
            tc.tile_pool(name="px", bufs=1, space="PSUM") as px,
            tc.tile_pool(name="psc", bufs=1, space="PSUM") as psc,
        ):
            # ---- allgather the node/weight payload ----
            if sim_nocc:
                ob = dpool.tile([TROWS, FEAT], bf16)
                nc.gpsimd.dma_start(ob[:], d_shard[:])
            else:
                ib = dpool.tile([SROWS, FEAT], bf16)
                ob = dpool.tile([TROWS, FEAT], bf16)
                nc.gpsimd.dma_start(ib[:], d_shard[:])
                base = mesh_index * MESHN
                nc.gpsimd.collective_compute(
                    "AllGather",
                    mybir.AluOpType.bypass,
                    replica_groups=[list(range(base, base + MESHN))],
                    ins=[ib.opt()],
                    outs=[ob.opt()],
                )
            wflat = ob[:].rearrange("r w -> (r w)")

            # ---- constants ----
            ident = cpool.tile([128, 128], bf16)
            make_identity(nc, ident[:])
            iota = cpool.tile([128, W], i32)
            nc.gpsimd.iota(iota[:], pattern=[[1, W]], base=0, channel_multiplier=0)
            w1sb = cpool.tile([128, 6 * HID], bf16)
            nc.sync.dma_start(
                out=w1sb[:].rearrange("p (c w) -> p c w", c=6),
                in_=wflat[W1OFF:W2OFF].rearrange("(c p w) -> p c w", p=128, w=HID),
            )
            w2sb = cpool.tile([128, 2 * DOUT], bf16)
            nc.sync.dma_start(
                out=w2sb[:].rearrange("p (c w) -> p c w", c=2),
                in_=wflat[W2OFF:B1OFF].rearrange("(c p w) -> p c w", p=128, w=DOUT),
            )
            b1sb = cpool.tile([128, 2], f32)
            nc.sync.dma_start(
                out=b1sb[:],
                in_=wflat[B1OFF:PAYW].rearrange("(p w) -> p w", p=128).bitcast(f32),
            )

            # ---- whole-run resident loads: meta, dist; resident output ----
            # chunked across queues so tile 0 starts early and transfers
            # overlap the pipeline
            meta_all = cpool.tile([SUB, T * MW], i32)
            meta_src = d_edges[0:SUB * T * MW].rearrange(
                "(p t w) -> p t w", p=SUB, w=MW
            )
            mchunk = -(-T // 4)
            for k in range(0, T, mchunk):
                ke = min(k + mchunk, T)
                nc.sync.dma_start(
                    out=meta_all[:].rearrange("p (t w) -> p t w", w=MW)[:, k:ke],
                    in_=meta_src[:, k:ke],
                )
            dist_all = cpool.tile([DIST, T * TP], f8e4)
            dist_src = (
                d_edges[SUB * T * MW:]
                .rearrange("(p w) -> p w", p=DIST)
                .bitcast(f8e4)
                .rearrange("p (t e) -> p t e", e=TP)
            )
            dchunk = -(-T // 8)
            for k in range(0, T, dchunk):
                ke = min(k + dchunk, T)
                nc.scalar.dma_start(
                    out=dist_all[:].rearrange("p (t e) -> p t e", e=TP)[:, k:ke],
                    in_=dist_src[:, k:ke],
                )
            out_all = cpool.tile([W, T * DOUT], bf16)
            OCH = 8  # output store chunk (tiles)

            def rot3(x_jkm, rote16, out_jkl, nj, eng):
                """out[j,k,l] = sum_m x[j,k,m] * rote[k,?,?] with the rote
                view supplied per direction.  x_jkm: AP [SUB, nj*L*2]
                (j,k,m); rote16: AP [SUB, 16] natural (k,m,l);
                out_jkl: AP [SUB, nj*L*2]."""
                t0 = pool3.tile([SUB, NREP * L * 2], bf16, tag="rt0")
                t1 = pool3.tile([SUB, NREP * L * 2], bf16, tag="rt1")
                xv = x_jkm.rearrange("p (j k m) -> p j k m", j=nj, k=L, m=2)
                # fwd: in1_m[p,k,l] = rote[k*4 + l*2 + m]  (strides k:4, l:2)
                rv = rote16.rearrange("p (k l m) -> p k l m", k=L, l=2, m=2)
                for m in range(2):
                    eng.tensor_tensor(
                        out=(t0 if m == 0 else t1)[:, : nj * L * 2].rearrange(
                            "p (j k l) -> p j k l", j=nj, k=L, l=2
                        ),
                        in0=xv[:, :, :, m : m + 1].broadcast_to([SUB, nj, L, 2]),
                        in1=rv[:, :, :, m].unsqueeze(1).broadcast_to([SUB, nj, L, 2]),
                        op=AL.mult,
                    )
                eng.tensor_tensor(
                    out=out_jkl,
                    in0=t0[:, : nj * L * 2],
                    in1=t1[:, : nj * L * 2],
                    op=AL.add,
                )

            def rot3_back(m_jkm, rote16, out_jkl, eng):
                """out[j,k,l] = sum_m msg[j,k,m] * rote[k,m,l] (natural)."""
                nj = NR
                t0 = pool3.tile([SUB, NR * L * 2], bf16, tag="bt0")
                t1 = pool3.tile([SUB, NR * L * 2], bf16, tag="bt1")
                xv = m_jkm.rearrange("p (j k m) -> p j k m", j=nj, k=L, m=2)
                rv = rote16.rearrange("p (k m l) -> p k m l", k=L, m=2, l=2)
                for m in range(2):
                    eng.tensor_tensor(
                        out=(t0 if m == 0 else t1)[:].rearrange(
                            "p (j k l) -> p j k l", j=nj, k=L, l=2
                        ),
                        in0=xv[:, :, :, m : m + 1].broadcast_to([SUB, nj, L, 2]),
                        in1=rv[:, :, m, :].unsqueeze(1).broadcast_to([SUB, nj, L, 2]),
                        op=AL.mult,
                    )
                eng.tensor_tensor(out=out_jkl, in0=t0[:], in1=t1[:], op=AL.add)

            def emit_front(t):
                # ---- per-tile views of the resident tiles ----
                meta = meta_all[:, t * MW:(t + 1) * MW]
                dist_sb = pool.tile([DIST, TP], bf16, tag="distbf")
                nc.scalar.activation(
                    out=dist_sb[:], in_=dist_all[:, t * TP:(t + 1) * TP],
                    func=ACTF.Copy,
                )
                ridx = meta[:, 0:4]
                crel = meta[:, 4:8]
                rote = meta[:, 8:40].bitcast(bf16)       # [128, 64]
                winr = meta[0:W, 40:41]

                # ---- one-hots (independent of gathers) ----
                onehot_e = pool.tile([SUB, NSUBT * W], bf16)
                onehot_w = pool.tile([W, TP], bf16)
                p_oh = ptr.tile([W, TP], bf16, tag="ptrans")
                nc.vector.tensor_tensor(
                    out=onehot_e[:].rearrange("p (s w) -> p s w", s=NSUBT),
                    in0=crel[:, 0:NSUBT].unsqueeze(-1).broadcast_to([SUB, NSUBT, W]),
                    in1=iota[:, :].unsqueeze(1).broadcast_to([SUB, NSUBT, W]),
                    op=AL.is_equal,
                )
                for s in range(NSUBT):
                    nc.tensor.transpose(
                        out=p_oh[:, s * SUB:(s + 1) * SUB],
                        in_=onehot_e[:, s * W:(s + 1) * W],
                        identity=ident[:],
                    )
                nc.scalar.activation(out=onehot_w[:], in_=p_oh[:], func=ACTF.Copy)

                # ---- gathers (from the allgathered table) ----
                win = pool.tile([W, FEAT], bf16)
                nc.gpsimd.indirect_dma_start(
                    out=win[:],
                    out_offset=None,
                    in_=ob[:],
                    in_offset=IndirectOffsetOnAxis(ap=winr, axis=0),
                )
                src_g = pool.tile([SUB, NSUBT * FEAT], bf16)
                for s in range(NSUBT):
                    nc.gpsimd.indirect_dma_start(
                        out=src_g[:, s * FEAT:(s + 1) * FEAT],
                        out_offset=None,
                        in_=ob[:],
                        in_offset=IndirectOffsetOnAxis(ap=ridx[:, s:s + 1], axis=0),
                    )

                featT = pool.tile([128, 3 * TP], bf16)
                sdst = pool.tile([SCAL, TP], bf16)
                ssrc = pool.tile([SCAL, TP], bf16)

                for s in range(NSUBT):
                    cL = s * SUB
                    rfs = rote[:, s * 16:(s + 1) * 16]

                    # ---- dst rot features: expand + rotate ----
                    p_x1 = px.tile([SUB, ROTF], f32, tag="px")
                    nc.tensor.matmul(
                        out=p_x1[:],
                        lhsT=onehot_w[:, cL:cL + SUB],
                        rhs=win[:, SCAL:FEAT],
                        start=True,
                        stop=True,
                    )
                    # xrot = [dst_rot 192 | src_rot 192] in one tile so the
                    # three 128-col transposes cover both
                    sg = src_g[:, s * FEAT:(s + 1) * FEAT]
                    xrot = pool3.tile([SUB, 2 * ROTF], bf16, tag="xrot")
                    rot3(p_x1[:], rfs, xrot[:, 0:ROTF], NREP, nc.vector)
                    rot3(sg[:, SCAL:FEAT], rfs, xrot[:, ROTF:2 * ROTF], NREP,
                         nc.vector)

                    # ---- transposes into chunk tiles ----
                    ptn = ptr.tile([128, 512], bf16, tag="ptrans")
                    for bb in range(3):
                        nc.tensor.transpose(
                            out=ptn[:, bb * 128:(bb + 1) * 128],
                            in_=xrot[:, bb * 128:(bb + 1) * 128],
                            identity=ident[:],
                        )
                    nc.tensor.transpose(
                        out=ptn[0:SCAL, 384:512], in_=sg[:, 0:SCAL], identity=ident[:]
                    )
                    # merged copy of the three 128-part sections -> featT blocks
                    nc.scalar.activation(
                        out=featT[:].rearrange("p (c e) -> p c e", c=3, e=TP)[
                            :, :, cL:cL + SUB
                        ],
                        in_=ptn[:, 0:384].rearrange("p (c e) -> p c e", c=3, e=SUB),
                        func=ACTF.Copy,
                    )
                    nc.scalar.activation(
                        out=ssrc[:, cL:cL + SUB], in_=ptn[0:SCAL, 384:512],
                        func=ACTF.Copy,
                    )

                # ---- dst scalar expand (once per tile) ----
                p_x2 = pph.tile([SCAL, TP], f32, tag="ph")
                nc.tensor.matmul(
                    out=p_x2[:],
                    lhsT=win[:, 0:SCAL],
                    rhs=onehot_w[:],
                    start=True,
                    stop=True,
                )
                nc.scalar.activation(out=sdst[:], in_=p_x2[:], func=ACTF.Copy)

                # ---- MLP layer 1 + relu ----
                rhs_chunks = [
                    featT[:, 0:TP], featT[:, TP:2 * TP], featT[:, 2 * TP:3 * TP],
                    sdst[:], ssrc[:], dist_sb[:],
                ]
                hT = pool.tile([128, 2 * TP], bf16)
                for hh in range(2):
                    p_h = pph.tile([128, TP], f32, tag="ph")
                    for c in range(6):
                        nc.tensor.matmul(
                            out=p_h[:],
                            lhsT=w1sb[0:KC[c], c * HID + hh * 128:c * HID + (hh + 1) * 128],
                            rhs=rhs_chunks[c][0:KC[c], :],
                            start=(c == 0),
                            stop=(c == 5),
                        )
                    nc.scalar.activation(
                        out=hT[:, hh * TP:(hh + 1) * TP],
                        in_=p_h[:],
                        func=ACTF.Relu,
                        bias=b1sb[:, hh:hh + 1],
                    )

                # ---- MLP layer 2 (2 partition chunks of 128/64) ----
                msgT = []
                for dd, (d0, dw) in enumerate([(0, 128), (128, 64)]):
                    p_o = ppo.tile([dw, TP], f32, tag="po")
                    for hh in range(2):
                        nc.tensor.matmul(
                            out=p_o[:],
                            lhsT=w2sb[:, hh * DOUT + d0:hh * DOUT + d0 + dw],
                            rhs=hT[:, hh * TP:(hh + 1) * TP],
                            start=(hh == 0),
                            stop=(hh == 1),
                        )
                    mt = pool.tile([dw, TP], bf16, tag=f"msgT{dd}")
                    if dd == 0:
                        nc.vector.tensor_copy(out=mt[:], in_=p_o[:])
                    else:
                        nc.scalar.activation(out=mt[:], in_=p_o[:], func=ACTF.Copy)
                    msgT.append(mt)

                return dict(rote=rote, onehot_e=onehot_e, msgT=msgT, t=t)

            def emit_back(st):
                rote = st["rote"]
                onehot_e = st["onehot_e"]
                msgT = st["msgT"]
                t = st["t"]
                # ---- back-rotation + segment sum ----
                p_sc = psc.tile([W, DOUT], f32, tag="psc")
                for s in range(NSUBT):
                    cL = s * SUB
                    rbs = rote[:, s * 16:(s + 1) * 16]
                    p_m = ptr.tile([128, DOUT], bf16, tag="ptrans")
                    nc.tensor.transpose(
                        out=p_m[:, 0:128], in_=msgT[0][:, cL:cL + SUB],
                        identity=ident[:],
                    )
                    nc.tensor.transpose(
                        out=p_m[:, 128:192], in_=msgT[1][:, cL:cL + SUB],
                        identity=ident[0:64, 0:64],
                    )
                    out_sb = pool3.tile([SUB, DOUT], bf16, tag="outsb")
                    nc.scalar.activation(
                        out=out_sb[:, 0:NS], in_=p_m[:, 0:NS], func=ACTF.Copy
                    )
                    rot3_back(p_m[:, NS:DOUT], rbs, out_sb[:, NS:DOUT], nc.vector)
                    nc.tensor.matmul(
                        out=p_sc[:],
                        lhsT=onehot_e[:, s * W:(s + 1) * W],
                        rhs=out_sb[:],
                        start=(s == 0),
                        stop=(s == NSUBT - 1),
                    )
                nc.scalar.activation(
                    out=out_all[:, t * DOUT:(t + 1) * DOUT], in_=p_sc[:],
                    func=ACTF.Copy,
                )
                # flush finished output chunk while the pipeline continues
                if (t + 1) % OCH == 0 or t == T - 1:
                    k = (t // OCH) * OCH
                    nc.sync.dma_start(
                        out=d_acc[:, k * DOUT:(t + 1) * DOUT],
                        in_=out_all[:, k * DOUT:(t + 1) * DOUT],
                    )

            # software pipeline: emit front(t+1) before back(t) so the
            # scheduler interleaves t+1's gathers/rotations with t's MLP
            st = emit_front(0)
            for t in range(1, T):
                st_next = emit_front(t)
                emit_back(st)
                st = st_next
            emit_back(st)

    nc.compile()
    return nc


_PROGRAM_CACHE = {}


def _get_program(T, mesh_index=0):
    key = (T, mesh_index)
    if key not in _PROGRAM_CACHE:
        _PROGRAM_CACHE[key] = _build_program(T, mesh_index=mesh_index)
    return _PROGRAM_CACHE[key]


class _PjrtExec:
    """Persistent jitted SPMD executables for one Bass program, run as NMESH
    concurrent disjoint meshes of MESHN cores each (axon/PJRT).  Per-execute
    dispatch overhead overlaps almost fully across the meshes."""

    def __init__(self, ncs):
        import jax
        from jax.sharding import Mesh, PartitionSpec, NamedSharding
        from jax.experimental.shard_map import shard_map
        import concourse.mybir as mybir
        from concourse.bass2jax import (
            _bass_exec_p,
            install_neuronx_cc_hook,
            partition_id_tensor,
        )

        install_neuronx_cc_hook()
        self.ncs = ncs
        nc0 = ncs[0]
        partition_name = (
            nc0.partition_id_tensor.name if nc0.partition_id_tensor else None
        )
        in_names, out_names, out_avals, zero_shapes = [], [], [], []
        for alloc in nc0.m.functions[0].allocations:
            if not isinstance(alloc, mybir.MemoryLocationSet):
                continue
            name = alloc.memorylocations[0].name
            if alloc.kind == "ExternalInput":
                if name != partition_name:
                    in_names.append(name)
            elif alloc.kind == "ExternalOutput":
                shape = tuple(alloc.tensor_shape)
                dtype = mybir.dt.np(alloc.dtype)
                out_names.append(name)
                out_avals.append(jax.core.ShapedArray(shape, dtype))
                zero_shapes.append((shape, dtype))
        self.in_names = in_names
        self.out_names = out_names
        self.out_avals = out_avals
        self.zero_shapes = zero_shapes
        n_params, n_outs = len(in_names), len(out_names)
        all_names = in_names + out_names
        if partition_name is not None:
            all_names.append(partition_name)
        donate = tuple(range(n_params, n_params + n_outs))

        def make_body(nc):
            def _body(*args):
                operands = list(args)
                if partition_name is not None:
                    operands.append(partition_id_tensor())
                outs = _bass_exec_p.bind(
                    *operands,
                    out_avals=tuple(out_avals),
                    in_names=tuple(all_names),
                    out_names=tuple(out_names),
                    lowering_input_output_aliases=(),
                    sim_require_finite=True,
                    sim_require_nnan=True,
                    nc=nc,
                )
                return tuple(outs)

            return _body

        devices = jax.devices()
        self.fns, self.shardings = [], []
        for mi in range(NMESH):
            mesh = Mesh(
                np.asarray(devices[mi * MESHN:(mi + 1) * MESHN]), ("core",)
            )
            self.fns.append(
                jax.jit(
                    shard_map(
                        make_body(ncs[mi]),
                        mesh=mesh,
                        in_specs=(PartitionSpec("core",),) * (n_params + n_outs),
                        out_specs=(PartitionSpec("core",),) * n_outs,
                        check_rep=False,
                    ),
                    donate_argnums=donate,
                    keep_unused=True,
                )
            )
            self.shardings.append(NamedSharding(mesh, PartitionSpec("core")))

    def stage_inputs(self, per_core_inputs):
        import jax

        staged = []
        for mi in range(NMESH):
            cores = range(mi * MESHN, (mi + 1) * MESHN)
            concat_in = [
                np.concatenate(
                    [np.asarray(per_core_inputs[c][n]) for c in cores], axis=0
                )
                for n in self.in_names
            ]
            staged.append(
                [jax.device_put(a, self.shardings[mi]) for a in concat_in]
            )
        return staged

    def fresh_zeros(self):
        return [
            [
                np.zeros((MESHN * s[0], *s[1:]), d)
                for (s, d) in self.zero_shapes
            ]
            for _ in range(NMESH)
        ]

    def put_zeros(self, zeros):
        import jax

        return [
            [jax.device_put(z, self.shardings[mi]) for z in zeros[mi]]
            for mi in range(NMESH)
        ]

    def launch(self, staged, zeros_dev):
        return [
            self.fns[mi](*staged[mi], *zeros_dev[mi]) for mi in range(NMESH)
        ]

    def run(self, staged, zeros):
        import jax

        outs = self.launch(staged, self.put_zeros(zeros))
        jax.block_until_ready(outs)
        return outs

    def results(self, outs):
        res = []
        for c in range(NCORES):
            mi, j = divmod(c, MESHN)
            res.append(
                {
                    n: np.asarray(outs[mi][i]).reshape(
                        MESHN, *self.out_avals[i].shape
                    )[j]
                    for i, n in enumerate(self.out_names)
                }
            )
        return res


_EXEC_CACHE = {}


def _get_exec(T):
    if T not in _EXEC_CACHE:
        _EXEC_CACHE[T] = _PjrtExec(
            [_get_program(T, mesh_index=mi) for mi in range(NMESH)]
        )
    return _EXEC_CACHE[T]


def kernel(**inputs):
    per_core_inputs, T, meta = _host_prep(inputs)
    ex = _get_exec(T)
    staged = ex.stage_inputs(per_core_inputs)
    outs = ex.run(staged, ex.fresh_zeros())
    return _assemble(ex.results(outs), meta)
